# revision 1
# baseline (speedup 1.0000x reference)
"""MixedArityTreeLSTM Trainium2 kernel.

Level-synchronous bottom-up Tree-LSTM over B=256 heap-indexed perfect binary
trees (511 nodes, depth 8), E=H=128. Pure data-parallel over 8 NeuronCores
(32 trees per core); all weights replicated.

Per-core layout: activations stored feature-major [H(part), nodes(free)].
Heap order makes left/right children the even/odd columns of the child level.
Binary/unary arity blending is folded into the matmuls via masked children:
    pre_g = W_g^T x + Ubt_g^T (m*h_l) + Ubb_g^T (m*h_r) + Uun_g^T ((1-m)*h_l)
            + m * (b_bin_g - b_un_g)   [K=1 outer-product matmul]
            + (bW_g + b_un_g)          [ACT bias]
Matmul operands are bf16 (2 col/cycle on PE); PSUM/h/c/gates stay fp32.
The embedding gather uses dma_gather(transpose=True) on a bf16 embedding
table, which lands x^T (feature-major) in SBUF directly.
"""

import os

import numpy as np
import ml_dtypes

# debug knobs (bisection); full kernel when unset
DBG_MIN_LVL = int(os.environ.get("TL_MIN_LVL", "0"))  # stop after this level
DBG_NO_DELTA = os.environ.get("TL_NO_DELTA", "") == "1"
N_QUEUES = int(os.environ.get("TL_NQ", "2"))

B, D = 256, 8
V, E, H = 32000, 128, 128
N_NODES = 2 ** (D + 1) - 1  # 511
NCORES = 8
BL = B // NCORES  # 32 trees per core

# levels in processing order: leaves (l=8) then 7..0
# (lvl, n real cols, Pw padded cols)
LEVELS = [(l, BL * (2**l), max(128, BL * (2**l))) for l in range(D, -1, -1)]
LVL_N = {l: BL * (2**l) for l in range(D + 1)}
LVL_PW = {l: max(128, BL * (2**l)) for l in range(D + 1)}

# chunks per level (chunk = up to 512 cols)
CPL = {l: max(1, LVL_N[l] // 512) for l in range(D + 1)}

# post-order dependency wave over the chunk tree: children before parent
ORDER = []


def _post(l, j):
    if l < D:
        if CPL[l + 1] == 2 * CPL[l]:
            _post(l + 1, 2 * j)
            _post(l + 1, 2 * j + 1)
        else:
            assert CPL[l + 1] == CPL[l] == 1
            _post(l + 1, 0)
    ORDER.append((l, j))


_post(0, 0)

# gather calls in wave order: (lvl, col0 within level's padded xT, width)
GATHER_CALLS = [
    (lvl, j * 512, min(512, LVL_PW[lvl] - j * 512)) for lvl, j in ORDER
]

# internal-level compute chunks in wave order: (cid, lvl, c0, N, mask offset)
CHUNKS = []
_moff = 0
for lvl, j in ORDER:
    if lvl == D:
        continue
    N = min(512, LVL_N[lvl] - j * 512)
    CHUNKS.append((len(CHUNKS), lvl, j * 512, N, _moff))
    _moff += N
N_MASK_ROWS = len(CHUNKS)  # 19
MASKB_LEN = _moff  # 8160

IDX_COLS = sum(w // 16 for _, _, w in GATHER_CALLS)  # 1032

BF16 = ml_dtypes.bfloat16

_CACHE = {}


def _build_nc():
    """Build the (SPMD, per-core) Bass/Tile kernel. Cached per process."""
    if "nc" in _CACHE:
        return _CACHE["nc"]

    from contextlib import ExitStack

    import concourse.mybir as mybir
    import concourse.tile as tile
    from concourse import bacc

    dt = mybir.dt
    AF = mybir.ActivationFunctionType

    nc = bacc.Bacc(num_swdge_queues=N_QUEUES)

    emb_d = nc.dram_tensor("emb_bf", [V, E], dt.bfloat16, kind="ExternalInput")
    idx_d = nc.dram_tensor("gidx", [128, IDX_COLS], dt.int16, kind="ExternalInput")
    mbc_d = nc.dram_tensor(
        "mbcast", [128, MASKB_LEN], dt.bfloat16, kind="ExternalInput"
    )
    maskb_d = nc.dram_tensor(
        "maskb", [1, MASKB_LEN], dt.bfloat16, kind="ExternalInput"
    )
    w_d = nc.dram_tensor("w_bf", [4, E, H], dt.bfloat16, kind="ExternalInput")
    ubt_d = nc.dram_tensor("ubt_bf", [5, H, H], dt.bfloat16, kind="ExternalInput")
    ubb_d = nc.dram_tensor("ubb_bf", [5, H, H], dt.bfloat16, kind="ExternalInput")
    uun_d = nc.dram_tensor("uun_bf", [4, H, H], dt.bfloat16, kind="ExternalInput")
    # bias rows: 0=b_leaf 1=bc_i 2=bc_fL 3=b_fR 4=bc_o 5=bc_u
    bias_d = nc.dram_tensor("biases", [6, H], dt.float32, kind="ExternalInput")
    # delta rows: 0=d_i 1=d_fL 2=d_o 3=d_u 4=+40 (f_r unary kill)
    delt_d = nc.dram_tensor("deltas", [5, H], dt.bfloat16, kind="ExternalInput")

    h_out_d = nc.dram_tensor("h_out", [H, BL], dt.float32, kind="ExternalOutput")
    c_out_d = nc.dram_tensor("c_out", [H, BL], dt.float32, kind="ExternalOutput")

    with tile.TileContext(nc) as tc, ExitStack() as ctx:
        consts = ctx.enter_context(tc.tile_pool(name="consts", bufs=1))

        w_sb = consts.tile([E, 4, H], dt.bfloat16)
        nc.sync.dma_start(out=w_sb, in_=w_d[:, :, :].rearrange("g e h -> e g h"))
        ubt_sb = consts.tile([H, 5, H], dt.bfloat16)
        nc.sync.dma_start(out=ubt_sb, in_=ubt_d[:, :, :].rearrange("g k h -> k g h"))
        ubb_sb = consts.tile([H, 5, H], dt.bfloat16)
        nc.sync.dma_start(out=ubb_sb, in_=ubb_d[:, :, :].rearrange("g k h -> k g h"))
        uun_sb = consts.tile([H, 4, H], dt.bfloat16)
        nc.sync.dma_start(out=uun_sb, in_=uun_d[:, :, :].rearrange("g k h -> k g h"))
        bias_sb = consts.tile([H, 6], dt.float32)
        nc.sync.dma_start(out=bias_sb, in_=bias_d[:, :].rearrange("n h -> h n"))
        delt_sb = consts.tile([1, 5, H], dt.bfloat16)
        nc.sync.dma_start(out=delt_sb, in_=delt_d[:, :].rearrange("(o g) h -> o g h", o=1))
        idx_sb = consts.tile([128, IDX_COLS], dt.int16)
        nc.sync.dma_start(out=idx_sb, in_=idx_d[:, :])
        mbc_sb = consts.tile([128, MASKB_LEN], dt.bfloat16)
        nc.sync.dma_start(out=mbc_sb, in_=mbc_d[:, :])
        maskb_sb = consts.tile([1, MASKB_LEN], dt.bfloat16)
        nc.sync.dma_start(out=maskb_sb, in_=maskb_d[:, :])

        # --- per-level xT tiles + all gathers issued up front ---
        lev = ctx.enter_context(tc.tile_pool(name="lev", bufs=1))
        xt = {}
        for lvl, n, pw in LEVELS:
            xt[lvl] = lev.tile(
                [128, pw], dt.bfloat16, name=f"xTl{lvl}", tag=f"xTl{lvl}"
            )


        # --- working pools ---
        psum = ctx.enter_context(tc.tile_pool(name="psum", bufs=8, space="PSUM"))
        work = ctx.enter_context(tc.tile_pool(name="work", bufs=2))

        h_t = {}
        c_t = {}
        n8 = LVL_N[D]
        h_t[D] = lev.tile([H, n8], dt.bfloat16, name="h_leaf", tag="h_leaf")

        # wave loop: for each chunk in post-order, gather its x then compute
        icols = {}
        _ic = 0
        for gi_, (lvl, c0, width) in enumerate(GATHER_CALLS):
            icols[(lvl, c0)] = (_ic, width, gi_)
            _ic += width // 16

        cid_of = {(lvl, c0): (cid, N, moff) for cid, lvl, c0, N, moff in CHUNKS}

        for lvl, j in ORDER:
            g0 = j * 512
            _icol, width, gi_ = icols[(lvl, g0)]
            out_view = xt[lvl][:, g0 : g0 + width].rearrange(
                "p (o n) -> p o n", o=1
            )
            nc.gpsimd.dma_gather(
                out_view,
                emb_d[:, :],
                idx_sb[:, _icol : _icol + width // 16],
                width,
                width,
                E,
                transpose=True,
                queue_num=gi_ % N_QUEUES,
            )

            if lvl == D:
                # leaf chunk: h = tanh(W3^T x + b3)
                ps = psum.tile([H, width], dt.float32, tag="pg", name="ps_leaf")
                nc.tensor.matmul(
                    ps, w_sb[:, 3, :], xt[D][:, g0 : g0 + width],
                    start=True, stop=True,
                )
                nc.scalar.activation(
                    h_t[D][:, g0 : g0 + width], ps, AF.Tanh, bias=bias_sb[:, 0:1]
                )
                continue

            cid, N, moff = cid_of[(lvl, g0)]
            c0 = g0
            if lvl < DBG_MIN_LVL:
                continue
            first_chunk = c0 == 0
            if first_chunk:
                n = LVL_N[lvl]
                hdt = dt.float32 if lvl == 0 else dt.bfloat16
                h_t[lvl] = lev.tile([H, n], hdt, name=f"h_l{lvl}", tag=f"h_l{lvl}")
                c_t[lvl] = lev.tile(
                    [H, n], dt.float32, name=f"c_l{lvl}", tag=f"c_l{lvl}"
                )

            hch = h_t[lvl + 1]
            pairs = hch[:, 2 * c0 : 2 * c0 + 2 * N].rearrange(
                "p (n two) -> p n two", two=2
            )
            h_e, h_o = pairs[:, :, 0], pairs[:, :, 1]

            mb = mbc_sb[:, moff : moff + N]

            heb = work.tile([128, N], dt.bfloat16, tag="heb", name="heb")
            nc.vector.tensor_mul(heb, h_e, mb)
            hob = work.tile([128, N], dt.bfloat16, tag="hob", name="hob")
            nc.vector.tensor_mul(hob, h_o, mb)
            heu = work.tile([128, N], dt.bfloat16, tag="heu", name="heu")
            nc.vector.tensor_sub(heu, h_e, heb)

            xs = xt[lvl][:, c0 : c0 + N]
            mrow = maskb_sb[:, moff : moff + N]
            top = lvl == D - 1  # children are leaves: c=0, skip f gates

            # gate -> (W idx, Ubin idx, Uun idx or None, delta idx or None)
            if top:
                gates = [("i", 0, 0, 0, 0), ("o", 2, 3, 2, 2), ("u", 3, 4, 3, 3)]
            else:
                gates = [
                    ("i", 0, 0, 0, 0),
                    ("fl", 1, 1, 1, 1),
                    ("fr", 1, 2, None, 4),
                    ("o", 2, 3, 2, 2),
                    ("u", 3, 4, 3, 3),
                ]

            pts = {}
            for gname, wi, ubi, uui, di in gates:
                ps = psum.tile([H, N], dt.float32, tag="pg", name=f"ps_{gname}")
                nc.tensor.matmul(ps, w_sb[:, wi, :], xs, start=True, stop=False)
                nc.tensor.matmul(ps, ubt_sb[:, ubi, :], heb, start=False, stop=False)
                nc.tensor.matmul(
                    ps, ubb_sb[:, ubi, :], hob,
                    start=False, stop=(DBG_NO_DELTA and uui is None),
                )
                if uui is not None:
                    nc.tensor.matmul(
                        ps, uun_sb[:, uui, :], heu, start=False, stop=DBG_NO_DELTA
                    )
                if not DBG_NO_DELTA:
                    nc.tensor.matmul(
                        ps, delt_sb[:, di, :], mrow, start=False, stop=True
                    )
                pts[gname] = ps

            gi = work.tile([128, N], dt.float32, tag="gi", name="gi")
            nc.scalar.activation(gi, pts["i"], AF.Sigmoid, bias=bias_sb[:, 1:2])
            go = work.tile([128, N], dt.float32, tag="go", name="go")
            nc.scalar.activation(go, pts["o"], AF.Sigmoid, bias=bias_sb[:, 4:5])
            gu = work.tile([128, N], dt.float32, tag="gu", name="gu")
            nc.scalar.activation(gu, pts["u"], AF.Tanh, bias=bias_sb[:, 5:6])

            cs = c_t[lvl][:, c0 : c0 + N]
            if top:
                nc.vector.tensor_mul(cs, gi, gu)
            else:
                gfl = work.tile([128, N], dt.float32, tag="gfl", name="gfl")
                nc.scalar.activation(
                    gfl, pts["fl"], AF.Sigmoid, bias=bias_sb[:, 2:3]
                )
                gfr = work.tile([128, N], dt.float32, tag="gfr", name="gfr")
                nc.scalar.activation(
                    gfr, pts["fr"], AF.Sigmoid, bias=bias_sb[:, 3:4]
                )
                cch = c_t[lvl + 1]
                cpairs = cch[:, 2 * c0 : 2 * c0 + 2 * N].rearrange(
                    "p (n two) -> p n two", two=2
                )
                c_e, c_o = cpairs[:, :, 0], cpairs[:, :, 1]

                t1 = work.tile([128, N], dt.float32, tag="t1", name="t1")
                nc.vector.tensor_mul(t1, gi, gu)
                t2 = work.tile([128, N], dt.float32, tag="t2", name="t2")
                nc.vector.tensor_mul(t2, gfl, c_e)
                nc.vector.tensor_add(cs, t1, t2)
                t3 = work.tile([128, N], dt.float32, tag="t3", name="t3")
                nc.vector.tensor_mul(t3, gfr, c_o)
                nc.vector.tensor_add(cs, cs, t3)

            tch = work.tile([128, N], dt.float32, tag="tch", name="tch")
            nc.scalar.activation(tch, cs, AF.Tanh)
            nc.vector.tensor_mul(h_t[lvl][:, c0 : c0 + N], go, tch)

        ol = DBG_MIN_LVL
        h_fin = h_t[ol][:, :BL]
        c_fin = c_t[ol][:, :BL] if ol in c_t else h_t[ol][:, :BL]
        eng = nc.sync if ol == 0 else nc.gpsimd
        eng.dma_start(out=h_out_d[:, :], in_=h_fin)
        eng.dma_start(out=c_out_d[:, :], in_=c_fin)

    nc.finalize()
    _CACHE["nc"] = nc
    return nc


def _wrap_idx(seg):
    """dma_gather index layout: unwrapped[i] = idxs[i % 16, i // 16],
    replicated across the 128 partitions."""
    w = seg.reshape(-1, 16).T.astype(np.int16)  # [16, len/16]
    return np.tile(w, (8, 1))


def prep_core_inputs(tokens_c, arity_c, shared):
    """Per-core input map. tokens_c [BL,511], arity_c [BL,255]."""
    idx_cols = []
    for lvl, c0, width in GATHER_CALLS:
        off, cnt = 2**lvl - 1, 2**lvl
        flat = np.asarray(tokens_c[:, off : off + cnt]).reshape(-1)
        pw = LVL_PW[lvl]
        if pw != flat.size:
            flat = np.concatenate([flat, np.zeros(pw - flat.size, np.int64)])
        idx_cols.append(_wrap_idx(flat[c0 : c0 + width]))
    gidx = np.concatenate(idx_cols, axis=1)
    assert gidx.shape == (128, IDX_COLS)

    maskb = np.zeros((1, MASKB_LEN), BF16)
    for cid, lvl, c0, N, moff in CHUNKS:
        off = 2**lvl - 1
        m = (
            (np.asarray(arity_c[:, off : off + 2**lvl]).reshape(-1) == 1)
            .astype(np.float32)
        )
        maskb[0, moff : moff + N] = m[c0 : c0 + N].astype(BF16)

    return dict(
        shared,
        gidx=gidx,
        maskb=maskb,
        mbcast=np.broadcast_to(maskb, (128, MASKB_LEN)).copy(),
    )


def prep_shared_inputs(emb, W, bW, Ubin, bUbin, Uun, bUun):
    emb = np.asarray(emb, np.float32)
    W = np.asarray(W, np.float32)
    bW = np.asarray(bW, np.float32)
    Ubin = np.asarray(Ubin, np.float32)
    bUbin = np.asarray(bUbin, np.float32)
    Uun = np.asarray(Uun, np.float32)
    bUun = np.asarray(bUun, np.float32)

    biases = np.stack(
        [
            bW[3],                # leaf
            bW[0] + bUun[0],      # i common
            bW[1] + bUun[1],      # fL common
            bW[1] + bUbin[2] - 40.0,  # fR (binary-only; -40 kills unary)
            bW[2] + bUun[2],      # o common
            bW[3] + bUun[3],      # u common
        ]
    ).astype(np.float32)
    deltas = np.stack(
        [
            bUbin[0] - bUun[0],
            bUbin[1] - bUun[1],
            bUbin[3] - bUun[2],
            bUbin[4] - bUun[3],
            np.full(H, 40.0, np.float32),
        ]
    ).astype(BF16)

    return dict(
        emb_bf=emb.astype(BF16),
        w_bf=W.astype(BF16),
        ubt_bf=Ubin[:, :H, :].astype(BF16),
        ubb_bf=Ubin[:, H:, :].astype(BF16),
        uun_bf=Uun.astype(BF16),
        biases=biases,
        deltas=deltas,
    )


def kernel(tokens, arity, emb, W, bW, Ubin, bUbin, Uun, bUun):
    from concourse.bass_utils import run_bass_kernel_spmd

    tokens = np.asarray(tokens)
    arity = np.asarray(arity)

    shared = prep_shared_inputs(emb, W, bW, Ubin, bUbin, Uun, bUun)
    in_maps = [
        prep_core_inputs(
            tokens[k * BL : (k + 1) * BL], arity[k * BL : (k + 1) * BL], shared
        )
        for k in range(NCORES)
    ]

    nc = _build_nc()
    res = run_bass_kernel_spmd(nc, in_maps, core_ids=list(range(NCORES)))
    results = res.results

    h = np.concatenate([r["h_out"].T for r in results], axis=0)
    c = np.concatenate([r["c_out"].T for r in results], axis=0)
    return h.astype(np.float32), c.astype(np.float32)



# revision 20
# speedup vs baseline: 1.3161x; 1.3161x over previous
"""MixedArityTreeLSTM Trainium2 kernel (v2).

Level-synchronous bottom-up Tree-LSTM over B=256 heap-indexed perfect binary
trees (511 nodes, depth 8), E=H=128. Pure data-parallel over 8 NeuronCores
(32 trees per core); weights replicated.

v2 redesign vs v1:
- Embedding gather done on HOST (numpy take); x streamed in as plain DMA.
- Per-level "split" column order: level l+1 is stored [left-children |
  right-children] of level l's column order, so every child read (h_e, h_o,
  c_e, c_o) is a contiguous slice and DVE mask-multiplies run in fast mode.
- Ubt' = Ubt - Uun folding: pre = W x + Ubt'·(m h_l) + Ubb·(m h_r) + Uun·h_l,
  so no (1-m) mask op is needed.
- Per-gate bias+mask-delta applied with one K=2 matmul ([b_g; db_g]^T
  [ones; m]) so gate activations need no ACT bias -> 4 sigmoid gates are
  activated in ONE batched ACT instruction over a packed PSUM tile.
- fr gate's unary kill: multiply c_o by the arity mask (Pool engine) instead
  of a +-40 bias hack.
- Elementwise c/h chain split across DVE and Pool engines.
"""

import numpy as np
import ml_dtypes

B, D = 256, 8
V, E, H = 32000, 128, 128
NCORES = 8
BL = B // NCORES  # 32 trees per core

LVLN = {l: BL * (2 ** l) for l in range(D + 1)}  # cols per level per core
N_INT = sum(LVLN[l] for l in range(D))  # 8160 internal cols
N_ALL = N_INT + LVLN[D]  # 16352

# x layout: leaf level first, then levels 7..0
XOFF = {}
_o = 0
for l in [D] + list(range(D - 1, -1, -1)):
    XOFF[l] = _o
    _o += LVLN[l]
# mask layout: levels 7..0
MOFF = {}
_o = 0
for l in range(D - 1, -1, -1):
    MOFF[l] = _o
    _o += LVLN[l]

WC = 256          # internal chunk width
WC_LEAF = 512     # leaf chunk width

BF16 = ml_dtypes.bfloat16

# split-order permutations: perm[l][p] = flat tree-major index (t*2^l + j)
PERM = {0: np.arange(BL, dtype=np.int64) * 1}
for l in range(D):
    e = PERM[l]
    t, j = e >> l, e & ((1 << l) - 1)
    left = (t << (l + 1)) + 2 * j
    PERM[l + 1] = np.concatenate([left, left + 1])

_CACHE = {}


def _build_nc():
    if "nc" in _CACHE:
        return _CACHE["nc"]

    from contextlib import ExitStack

    import concourse.mybir as mybir
    import concourse.tile as tile
    from concourse import bacc

    dt = mybir.dt
    AF = mybir.ActivationFunctionType

    nc = bacc.Bacc()

    xall_d = nc.dram_tensor("xall", [128, N_ALL], dt.bfloat16, kind="ExternalInput")
    mbc_d = nc.dram_tensor("mbc", [128, N_INT], dt.bfloat16, kind="ExternalInput")
    onesm_d = nc.dram_tensor("onesm", [2, N_INT], dt.bfloat16, kind="ExternalInput")
    wq_d = nc.dram_tensor("wq", [E, 4, H], dt.bfloat16, kind="ExternalInput")
    ubtp_d = nc.dram_tensor("ubtp", [H, 5, H], dt.bfloat16, kind="ExternalInput")
    ubb_d = nc.dram_tensor("ubb", [H, 5, H], dt.bfloat16, kind="ExternalInput")
    uun_d = nc.dram_tensor("uun", [H, 4, H], dt.bfloat16, kind="ExternalInput")
    bd_d = nc.dram_tensor("bd", [2, 5, H], dt.bfloat16, kind="ExternalInput")
    bleaf_d = nc.dram_tensor("bleaf", [H, 1], dt.float32, kind="ExternalInput")

    h_out_d = nc.dram_tensor("h_out", [H, BL], dt.float32, kind="ExternalOutput")
    c_out_d = nc.dram_tensor("c_out", [H, BL], dt.float32, kind="ExternalOutput")

    with tile.TileContext(nc) as tc, ExitStack() as ctx:
        consts = ctx.enter_context(tc.tile_pool(name="consts", bufs=1))

        wq = consts.tile([E, 4, H], dt.bfloat16)
        ubtp = consts.tile([H, 5, H], dt.bfloat16)
        ubb = consts.tile([H, 5, H], dt.bfloat16)
        uun = consts.tile([H, 4, H], dt.bfloat16)
        bd = consts.tile([2, 5, H], dt.bfloat16)
        bleaf = consts.tile([H, 1], dt.float32)

        xall = consts.tile([128, N_ALL], dt.bfloat16, name="xall", tag="xall")
        mbc = consts.tile([128, N_INT], dt.bfloat16, name="mbc", tag="mbc")
        onesm = consts.tile([2, N_INT], dt.bfloat16)

        def dx(eng, t, d, a, b):
            eng.dma_start(out=t[:, a:b], in_=d[:, a:b])

        # All DMA issue work on SP + Pool so the ACT engine stays free for
        # activations. Leaf x + W first so PE can start; L7 masks + x next.
        nc.scalar.dma_start(out=bleaf, in_=bleaf_d[:, :])
        dx(nc.sync, xall, xall_d, 0, 2048)
        nc.sync.dma_start(out=wq, in_=wq_d[:, :, :])
        dx(nc.gpsimd, xall, xall_d, 8192, 12288)
        dx(nc.sync, mbc, mbc_d, 0, 1024)
        dx(nc.sync, onesm, onesm_d, 0, 1024)
        nc.sync.dma_start(out=ubtp, in_=ubtp_d[:, :, :])
        dx(nc.gpsimd, xall, xall_d, 4096, 6144)
        dx(nc.sync, xall, xall_d, 2048, 4096)
        nc.sync.dma_start(out=ubb, in_=ubb_d[:, :, :])
        nc.sync.dma_start(out=uun, in_=uun_d[:, :, :])
        nc.sync.dma_start(out=bd, in_=bd_d[:, :, :])
        dx(nc.gpsimd, xall, xall_d, 6144, 8192)
        dx(nc.sync, mbc, mbc_d, 1024, 2048)
        dx(nc.sync, onesm, onesm_d, 1024, 2048)
        dx(nc.sync, mbc, mbc_d, 2048, 4096)
        dx(nc.sync, onesm, onesm_d, 2048, 4096)
        dx(nc.sync, mbc, mbc_d, 4096, N_INT)
        dx(nc.sync, onesm, onesm_d, 4096, N_INT)
        dx(nc.sync, xall, xall_d, 12288, N_ALL)

        lev = ctx.enter_context(tc.tile_pool(name="lev", bufs=1))
        h_t, c_t = {}, {}
        h_t[D] = lev.tile([H, LVLN[D]], dt.bfloat16, name="h8", tag="h8")
        for l in range(D - 1, 0, -1):
            h_t[l] = lev.tile([H, LVLN[l]], dt.bfloat16, name=f"h{l}", tag=f"h{l}")
            c_t[l] = lev.tile([H, LVLN[l]], dt.float32, name=f"c{l}", tag=f"c{l}")
        h_t[0] = lev.tile([H, BL], dt.float32, name="h0", tag="h0")
        c_t[0] = lev.tile([H, BL], dt.float32, name="c0", tag="c0")

        psL = ctx.enter_context(tc.tile_pool(name="psL", bufs=2, space="PSUM"))
        psG = ctx.enter_context(tc.tile_pool(name="psG", bufs=2, space="PSUM"))
        work = ctx.enter_context(tc.tile_pool(name="work", bufs=3))

        # gate specs: (region, w_idx, ubtp_idx, ubb_idx, uun_idx|None, bd_idx)
        G_FULL = [
            (0, 0, 0, 0, 0, 0),   # i
            (1, 1, 1, 1, 1, 1),   # f_l
            (2, 1, 2, 2, None, 2),  # f_r (no unary path; c_o masked instead)
            (3, 2, 3, 3, 2, 3),   # o
            (4, 3, 4, 4, 3, 4),   # u
        ]
        G_TOP = [
            (0, 0, 0, 0, 0, 0),   # i
            (1, 2, 3, 3, 2, 3),   # o
            (2, 3, 4, 4, 3, 4),   # u
        ]

        import os as _os
        WARM_N = int(_os.environ.get("TL_WARM", "12"))

        def emit_leaf(k):
            """Leaf chunk k: h = tanh(W3^T x + b3) over cols [512k, 512k+512)."""
            s = slice(k * WC_LEAF, (k + 1) * WC_LEAF)
            ps = psL.tile([H, WC_LEAF], dt.float32, tag="psl", name="psl")
            nc.tensor.matmul(ps, wq[:, 3, :], xall[:, s], start=True, stop=True)
            nc.scalar.activation(h_t[D][:, s], ps, AF.Tanh, bias=bleaf[:, 0:1])

        def emit_chunk(l, c0, wc):
            N = LVLN[l]
            top = l == D - 1
            gates = G_TOP if top else G_FULL
            nsig = 2 if top else 4
            iu_, io_, iuu_ = (0, 1, 2) if top else (0, 3, 4)
            hch, cch = h_t[l + 1], (None if top else c_t[l + 1])

            xs = slice(XOFF[l] + c0, XOFF[l] + c0 + wc)
            ms = slice(MOFF[l] + c0, MOFF[l] + c0 + wc)
            le = slice(c0, c0 + wc)            # left child cols
            ro = slice(N + c0, N + c0 + wc)    # right child cols
            ls = slice(c0, c0 + wc)            # this level's cols

            hm = work.tile([128, 1024], dt.bfloat16, tag="hm", name="hm")
            hm = hm.rearrange("p (g n) -> p g n", n=wc)
            nc.vector.tensor_mul(hm[:, 0, :], hch[:, le], mbc[:, ms])
            nc.vector.tensor_mul(hm[:, 1, :], hch[:, ro], mbc[:, ms])
            if not top:
                com = work.tile([128, 512], dt.float32, tag="com", name="com")
                com = com[:, 0:wc]
                nc.gpsimd.tensor_mul(com, cch[:, ro], mbc[:, ms])

            # flat 3-bank PSUM tile viewed as ngate x wc regions
            psf = psG.tile([H, 1536], dt.float32, tag="psg", name="psg")
            ps = psf.rearrange("p (g n) -> p g n", n=wc)
            # When gate regions are bank-aligned (wc=512), issue all x-only
            # matmuls first: they have no h dependency, so the PE can chew
            # them while the previous level's activation tail drains.
            # (Non-bank-aligned regions share a PSUM zero region, which
            # forbids concurrently open accumulation groups.)
            wfirst = (wc * 4) % 2048 == 0
            if wfirst:
                for r, wi, ti, bi, ui, di in gates:
                    nc.tensor.matmul(ps[:, r, :], wq[:, wi, :], xall[:, xs],
                                     start=True, stop=False)
            for r, wi, ti, bi, ui, di in gates:
                po = ps[:, r, :]
                if not wfirst:
                    nc.tensor.matmul(po, wq[:, wi, :], xall[:, xs],
                                     start=True, stop=False)
                nc.tensor.matmul(po, ubtp[:, ti, :], hm[:, 0, :],
                                 start=False, stop=False)
                nc.tensor.matmul(po, ubb[:, bi, :], hm[:, 1, :],
                                 start=False, stop=False)
                if ui is not None:
                    nc.tensor.matmul(po, uun[:, ui, :], hch[:, le],
                                     start=False, stop=False)
                nc.tensor.matmul(
                    po, bd[:, di, :], onesm[:, ms], start=False, stop=True
                )

            gs = work.tile([128, 2560], dt.float32, tag="gs", name="gs")
            gs = gs.rearrange("p (g n) -> p g n", n=wc)
            nc.scalar.activation(gs[:, 0:nsig, :], ps[:, 0:nsig, :], AF.Sigmoid)
            nc.scalar.activation(gs[:, iuu_, :], ps[:, iuu_, :], AF.Tanh)

            cs = c_t[l][:, ls]
            if top:
                nc.vector.tensor_mul(cs, gs[:, iu_, :], gs[:, iuu_, :])
            else:
                t1 = work.tile([128, 512], dt.float32, tag="t1", name="t1")[:, 0:wc]
                nc.vector.tensor_mul(t1, gs[:, 0, :], gs[:, 4, :])
                t2 = work.tile([128, 512], dt.float32, tag="t2", name="t2")[:, 0:wc]
                nc.gpsimd.tensor_mul(t2, gs[:, 1, :], cch[:, le])
                t3 = work.tile([128, 512], dt.float32, tag="t3", name="t3")[:, 0:wc]
                nc.vector.tensor_mul(t3, gs[:, 2, :], com)
                a1 = work.tile([128, 512], dt.float32, tag="a1", name="a1")[:, 0:wc]
                nc.gpsimd.tensor_add(a1, t1, t2)
                nc.vector.tensor_add(cs, a1, t3)

            tch = work.tile([128, 512], dt.float32, tag="tch", name="tch")[:, 0:wc]
            nc.scalar.activation(tch, cs, AF.Tanh)
            nc.vector.tensor_mul(h_t[l][:, ls], gs[:, io_, :], tch)

        # leaf + L7 software-pipelined: leaf chunks in (left, right) pair
        # order so L7 chunk k only needs the leaf pair k
        for k in range(8):
            emit_leaf(k)
            emit_leaf(8 + k)
            emit_chunk(D - 1, 512 * k, 512)

        for l in range(D - 2, -1, -1):
            # keep-warm: filler matmuls so the PE p-state clock stays hot
            # across the serial dependency tail of the small levels
            if l <= 4 and WARM_N:
                wps = psL.tile([H, WC_LEAF], dt.float32, tag="psl", name="warm")
                for _ in range(WARM_N):
                    nc.tensor.matmul(
                        wps[:, 0:256], wq[:, 0, :], xall[:, 0:256],
                        start=True, stop=True, skip_group_check=True,
                    )
            N = LVLN[l]
            wc = min(WC, N)
            for c0 in range(0, N, wc):
                emit_chunk(l, c0, wc)

        nc.sync.dma_start(out=h_out_d[:, :], in_=h_t[0][:, :])
        nc.sync.dma_start(out=c_out_d[:, :], in_=c_t[0][:, :])

    nc.finalize()
    _CACHE["nc"] = nc
    return nc


def prep_shared_inputs(emb, W, bW, Ubin, bUbin, Uun, bUun):
    emb = np.asarray(emb, np.float32)
    W = np.asarray(W, np.float32)
    bW = np.asarray(bW, np.float32)
    Ubin = np.asarray(Ubin, np.float32)
    bUbin = np.asarray(bUbin, np.float32)
    Uun = np.asarray(Uun, np.float32)
    bUun = np.asarray(bUun, np.float32)

    ubt = Ubin[:, :H, :]  # [5, H, H] top half (left child)
    ubb_ = Ubin[:, H:, :]  # bottom half (right child)
    # Ubt' = Ubt - Uun for gates with a unary path (i, fl, o, u)
    ubtp = ubt.copy()
    for gi_, ui_ in ((0, 0), (1, 1), (3, 2), (4, 3)):
        ubtp[gi_] = ubt[gi_] - Uun[ui_]

    # bias rows [b_g; db_g] per gate (i, fl, fr, o, u)
    bcom = np.stack([
        bW[0] + bUun[0],
        bW[1] + bUun[1],
        bW[1] + bUbin[2],
        bW[2] + bUun[2],
        bW[3] + bUun[3],
    ])
    bdel = np.stack([
        bUbin[0] - bUun[0],
        bUbin[1] - bUun[1],
        np.zeros(H, np.float32),
        bUbin[3] - bUun[2],
        bUbin[4] - bUun[3],
    ])
    bd = np.stack([bcom, bdel]).astype(BF16)  # [2, 5, H]

    return dict(
        emb_bf=emb.astype(BF16),
        wq=np.ascontiguousarray(W.transpose(1, 0, 2)).astype(BF16),
        ubtp=np.ascontiguousarray(ubtp.transpose(1, 0, 2)).astype(BF16),
        ubb=np.ascontiguousarray(ubb_.transpose(1, 0, 2)).astype(BF16),
        uun=np.ascontiguousarray(Uun.transpose(1, 0, 2)).astype(BF16),
        bd=bd,
        bleaf=bW[3].reshape(H, 1).astype(np.float32),
    )


def prep_core_inputs(tokens_c, arity_c, shared):
    """Per-core inputs. tokens_c [BL,511], arity_c [BL,255]."""
    tokens_c = np.asarray(tokens_c)
    arity_c = np.asarray(arity_c)
    emb_bf = shared["emb_bf"]

    xcols = np.empty((N_ALL, E), dtype=BF16)
    mrow = np.empty(N_INT, dtype=np.float32)
    for l in [D] + list(range(D - 1, -1, -1)):
        off, cnt = 2 ** l - 1, 2 ** l
        toks = tokens_c[:, off : off + cnt].reshape(-1)[PERM[l]]
        xcols[XOFF[l] : XOFF[l] + LVLN[l]] = emb_bf[toks]
        if l < D:
            ar = arity_c[:, off : off + cnt].reshape(-1)[PERM[l]]
            mrow[MOFF[l] : MOFF[l] + LVLN[l]] = (ar == 1).astype(np.float32)

    m16 = mrow.astype(BF16)
    onesm = np.stack([np.ones(N_INT, BF16), m16])  # [2, N_INT]
    out = {k: v for k, v in shared.items() if k != "emb_bf"}
    out["xall"] = np.ascontiguousarray(xcols.T)
    out["mbc"] = np.broadcast_to(m16, (128, N_INT)).copy()
    out["onesm"] = onesm
    return out


def kernel(tokens, arity, emb, W, bW, Ubin, bUbin, Uun, bUun):
    from concourse.bass_utils import run_bass_kernel_spmd

    tokens = np.asarray(tokens)
    arity = np.asarray(arity)

    shared = prep_shared_inputs(emb, W, bW, Ubin, bUbin, Uun, bUun)
    in_maps = [
        prep_core_inputs(
            tokens[k * BL : (k + 1) * BL], arity[k * BL : (k + 1) * BL], shared
        )
        for k in range(NCORES)
    ]

    nc = _build_nc()
    res = run_bass_kernel_spmd(nc, in_maps, core_ids=list(range(NCORES)))
    results = res.results

    h = np.concatenate([r["h_out"].T for r in results], axis=0)
    c = np.concatenate([r["c_out"].T for r in results], axis=0)
    return h.astype(np.float32), c.astype(np.float32)
